# revision 2
# baseline (speedup 1.0000x reference)
# BitStackLinear Trainium2 kernel (8-core column-parallel), v2.
#
# reference computation:
#   sign  = unpack_bits(qweight) in {-1,+1}            [4, 4096, 4096]  (b, o, i)
#   w     = sum_b sign_b * (u_b @ vt_b)                [4096, 4096]     (o, i)
#   out   = x @ w.T                                    [4, 2048, 4096]
#
# Sharding: column-parallel over out_features (512 per core). x replicated.
#
# v2 formation pipeline (per 128-row i-tile, all engines balanced):
#   PE    : 4 low-rank matmuls L_b.T[i,o] = vt_b.T @ u_b (K=16, N=512)
#           into two 2-bank f32 psum tiles (planes 01 / 23)
#   Scalar: 2 ACTIVATE copies psum f32 -> fp16 ls[128, 2048]
#   DVE   : 2 tensor_scalar ops (AND,SHL — 4x perf mode) expand host-packed
#           sign words (2 signs/word at bits 15/7) into {0,0x8000} masks,
#           1 wide XOR (prods = ls ^ masks, exact +-L), final 512-add
#   GpSimd: first-level 1024-wide plane add
#   PE    : token-group-0 matmuls (4 psum banks) keep the PE dense & warm
# Then remaining token groups: out[t, o] = sum_i xT[i,t].T @ wT[i,o] at the
# N=512 streaming roofline (~216 ns/matmul), flushes alternating Scalar/DVE.
#
# Host prep: transpose x to [in_f, tokens]; pack INVERTED sign bits 2-per-
# uint16-word: word[i, b*256+j] = inv(b, o=j)<<15 | inv(b, o=j+256)<<7 so
# bit-extraction is 2 DVE ops and masks land contiguous in (b, o).

import sys

import numpy as np

for p in ("/opt/trn_rl_repo", "/opt/pypackages"):
    if p not in sys.path:
        sys.path.insert(0, p)

import concourse.bacc as bacc
import concourse.mybir as mybir
import concourse.tile as tile
from concourse.bass_utils import run_bass_kernel_spmd

W_BIT, OUT_F, IN_F, K = 4, 4096, 4096, 16
B, S = 4, 2048
T = B * S                      # 8192 tokens
NCORES = 8
OS = OUT_F // NCORES           # 512 out features per core
N_ITILES = IN_F // 128         # 32
HALF = OS // 2                 # 256

# token groups: (start_token, n_ttiles). group 0 runs under formation with 4
# psum banks (the other 4 hold the in-flight low-rank psums); the rest use 8.
GROUPS = [(0, 4)] + [(512 + 1024 * g, 8) for g in range(7)] + [(7680, 4)]

FP16 = mybir.dt.float16
F32 = mybir.dt.float32
U16 = mybir.dt.uint16
Alu = mybir.AluOpType

_cached = {}


def build_nc():
    nc = bacc.Bacc("TRN2", target_bir_lowering=False, debug=False,
                   num_devices=NCORES)
    xt_p = nc.dram_tensor("xt", [IN_F, T], FP16, kind="ExternalInput").ap()
    qp_p = nc.dram_tensor("qp2", [IN_F, W_BIT * HALF], U16,
                          kind="ExternalInput").ap()
    ut_p = nc.dram_tensor("ut", [W_BIT, K, OS], FP16, kind="ExternalInput").ap()
    vt_p = nc.dram_tensor("vt4", [W_BIT, K, IN_F], FP16, kind="ExternalInput").ap()
    out_p = nc.dram_tensor("out", [T, OS], FP16, kind="ExternalOutput").ap()

    with tile.TileContext(nc) as tc:
        with (
            tc.tile_pool(name="const", bufs=1) as cpool,
            tc.tile_pool(name="wt", bufs=1) as wtpool,
            tc.tile_pool(name="fw", bufs=3) as fw,
            tc.tile_pool(name="fls", bufs=2) as fls,
            tc.tile_pool(name="fmk", bufs=2) as fmk,
            tc.tile_pool(name="fpr", bufs=2) as fpr,
            tc.tile_pool(name="fp01", bufs=2) as fp01,
            tc.tile_pool(name="mx", bufs=8) as mx,
            tc.tile_pool(name="mo", bufs=8) as mo,
        ):
            # resident operands
            vt_b = []
            ut_b = []
            for b in range(W_BIT):
                v = cpool.tile([K, IN_F], FP16, tag=f"vt{b}")
                nc.sync.dma_start(v[:], vt_p[b, :, :])
                vt_b.append(v)
                uu = cpool.tile([K, OS], FP16, tag=f"ut{b}")
                nc.sync.dma_start(uu[:], ut_p[b, :, :])
                ut_b.append(uu)

            # w.T tiles, one per i-tile
            wts = [
                wtpool.tile([128, OS], FP16, tag=f"wt{it}", name=f"wt_{it}")
                for it in range(N_ITILES)
            ]

            def mm_group(gi, it):
                t0, ntt = GROUPS[gi]
                xs = mx.tile([128, ntt * 128], FP16, tag="x")
                nc.sync.dma_start(
                    xs[:], xt_p[it * 128:(it + 1) * 128, t0:t0 + ntt * 128]
                )
                for tt in range(ntt):
                    nc.tensor.matmul(
                        acc_tiles[tt][:],
                        xs[:, tt * 128:(tt + 1) * 128],
                        wts[it][:],
                        start=(it == 0),
                        stop=(it == N_ITILES - 1),
                    )

            def flush_group(gi):
                t0, ntt = GROUPS[gi]
                for tt in range(ntt):
                    ot = mo.tile([128, OS], FP16, tag="o")
                    if tt % 2 == 0:
                        nc.scalar.copy(ot[:], acc_tiles[tt][:])
                    else:
                        nc.vector.tensor_copy(ot[:], acc_tiles[tt][:])
                    r0 = t0 + tt * 128
                    nc.sync.dma_start(out_p[r0:r0 + 128, :], ot[:])

            # ---- formation (per i-tile) pipelined with token group 0 ----
            with (
                tc.tile_pool(name="mps0", bufs=4, space="PSUM") as mps0,
                tc.tile_pool(name="psL", bufs=2, space="PSUM") as psL,
            ):
                acc_tiles = [
                    mps0.tile([128, OS], F32, tag="ps", name=f"acc_0_{tt}")
                    for tt in range(GROUPS[0][1])
                ]
                for it in range(N_ITILES):
                    isl = slice(it * 128, it * 128 + 128)

                    # packed sign words for this i-tile
                    wq = fw.tile([128, W_BIT * HALF], U16, tag="wq",
                                 name=f"wq_{it}")
                    nc.sync.dma_start(wq[:], qp_p[isl, :])

                    # low-rank psums, planes 01 -> plA, planes 23 -> plB
                    plA = psL.tile([128, 2 * OS], F32, tag="pl",
                                   name=f"plA_{it}")
                    nc.tensor.matmul(plA[:, 0:OS], vt_b[0][:, isl], ut_b[0][:],
                                     start=True, stop=True)
                    nc.tensor.matmul(plA[:, OS:2 * OS], vt_b[1][:, isl],
                                     ut_b[1][:], start=True, stop=True)
                    plB = psL.tile([128, 2 * OS], F32, tag="pl",
                                   name=f"plB_{it}")
                    nc.tensor.matmul(plB[:, 0:OS], vt_b[2][:, isl], ut_b[2][:],
                                     start=True, stop=True)
                    nc.tensor.matmul(plB[:, OS:2 * OS], vt_b[3][:, isl],
                                     ut_b[3][:], start=True, stop=True)

                    # evacuate to fp16 (Scalar)
                    ls = fls.tile([128, W_BIT * OS], FP16, tag="ls")
                    nc.scalar.copy(ls[:, 0:2 * OS], plA[:])
                    nc.scalar.copy(ls[:, 2 * OS:4 * OS], plB[:])

                    # sign masks in {0, 0x8000}: 2 wide 4x-mode DVE ops
                    mk = fmk.tile([128, W_BIT * OS], U16, tag="mk")
                    wq4 = wq[:].rearrange("p (b j) -> p b j", b=W_BIT)
                    mk4 = mk[:].rearrange("p (b h j) -> p b h j", b=W_BIT, h=2)
                    nc.vector.tensor_scalar(
                        mk4[:, :, 0, :], wq4, 0x8000, 0,
                        op0=Alu.bitwise_and, op1=Alu.logical_shift_left,
                    )
                    nc.vector.tensor_scalar(
                        mk4[:, :, 1, :], wq4, 0x0080, 8,
                        op0=Alu.bitwise_and, op1=Alu.logical_shift_left,
                    )

                    # prods = ls ^ masks (flips fp16 sign bit -> exact +-L)
                    pr = fpr.tile([128, W_BIT * OS], FP16, tag="pr")
                    nc.vector.tensor_tensor(
                        pr[:].bitcast(U16), ls[:].bitcast(U16), mk[:],
                        op=Alu.bitwise_xor,
                    )

                    # wT = (p0+p2) + (p1+p3): wide add on GpSimd, final on DVE
                    p01 = fp01.tile([128, 2 * OS], FP16, tag="p01")
                    nc.gpsimd.tensor_add(
                        p01[:], pr[:, 0:2 * OS], pr[:, 2 * OS:4 * OS]
                    )
                    nc.vector.tensor_add(
                        wts[it][:], p01[:, 0:OS], p01[:, OS:2 * OS]
                    )
                    mm_group(0, it)
                flush_group(0)

            # ---- remaining token groups (full 8 psum banks) ----
            with tc.tile_pool(name="mps", bufs=8, space="PSUM") as mps:
                for gi in range(1, len(GROUPS)):
                    acc_tiles = [
                        mps.tile([128, OS], F32, tag="ps", name=f"acc_{gi}_{tt}")
                        for tt in range(GROUPS[gi][1])
                    ]
                    for it in range(N_ITILES):
                        mm_group(gi, it)
                    flush_group(gi)
    nc.compile()
    return nc


def prep_inputs(x, qweight, u, vt):
    """Host-side shard prep. Returns per-core input maps."""
    x = np.asarray(x, dtype=np.float16)
    qweight = np.asarray(qweight)
    u = np.asarray(u, dtype=np.float16)
    vt = np.ascontiguousarray(np.asarray(vt, dtype=np.float16))

    xt = np.ascontiguousarray(x.reshape(T, IN_F).T)  # [IN_F, T]

    # unpack bits: (b, o, i); INVERT so mask=0x8000 <=> sign -1 (bit 0)
    bytes_ = qweight.astype(np.uint8)
    bits = np.unpackbits(bytes_.reshape(W_BIT, -1, 1), axis=2, bitorder="little")
    bits = bits.reshape(W_BIT, OUT_F, IN_F)
    inv = (1 - bits).astype(np.uint16)
    # per core c: word[i, b*HALF + j] = inv(b, o=c*OS+j, i)<<15
    #                                 | inv(b, o=c*OS+HALF+j, i)<<7
    iv = inv.reshape(W_BIT, NCORES, 2, HALF, IN_F)  # [b, c, h, j, i]
    words = (iv[:, :, 0] << np.uint16(15)) | (iv[:, :, 1] << np.uint16(7))
    # [b, c, j, i] -> [c, i, b, j]
    qp_all = words.transpose(1, 3, 0, 2)
    qp_all = np.ascontiguousarray(qp_all).reshape(NCORES, IN_F, W_BIT * HALF)

    in_maps = []
    for c in range(NCORES):
        uc = u[:, c * OS:(c + 1) * OS, :]                 # [4, 512, 16]
        ut = np.ascontiguousarray(uc.transpose(0, 2, 1))  # [4, 16, 512]
        in_maps.append({"xt": xt, "qp2": qp_all[c], "ut": ut, "vt4": vt})
    return in_maps


def kernel(x, qweight, u, vt, _trace=False):
    if "nc" not in _cached:
        _cached["nc"] = build_nc()
    nc = _cached["nc"]
    in_maps = prep_inputs(x, qweight, u, vt)
    res = run_bass_kernel_spmd(nc, in_maps, list(range(NCORES)), trace=_trace)
    _cached["last_result"] = res
    out = np.concatenate([res.results[c]["out"] for c in range(NCORES)], axis=1)
    return out.reshape(B, S, OUT_F).astype(np.float16)


# revision 11
# speedup vs baseline: 1.0355x; 1.0355x over previous
# BitStackLinear Trainium2 kernel (8-core column-parallel), v2.
#
# reference computation:
#   sign  = unpack_bits(qweight) in {-1,+1}            [4, 4096, 4096]  (b, o, i)
#   w     = sum_b sign_b * (u_b @ vt_b)                [4096, 4096]     (o, i)
#   out   = x @ w.T                                    [4, 2048, 4096]
#
# Sharding: column-parallel over out_features (512 per core). x replicated.
#
# v3 formation pipeline (per 128-row i-tile; GpSimd unused — its SBUF
# traffic starves the DVE):
#   PE    : 4 low-rank matmuls L_b.T[i,o] = vt_b.T @ u_b (K=16, N=512)
#           into two 2-bank f32 psum tiles (planes 01 / 23)
#   Scalar: 2 ACTIVATE copies psum f32 -> fp16 ls[128, 2048]
#   DVE   : 1 wide XOR with host-precomputed {0,0x8000} masks (exact +-L),
#           then the 1024- and 512-wide plane adds
#   PE    : token-group-0 matmuls (4 psum banks) keep the PE dense & warm
# Then remaining token groups: out[t, o] = sum_i xT[i,t].T @ wT[i,o] at the
# N=512 streaming roofline (~216 ns/matmul), flushes alternating Scalar/DVE.
#
# Host prep: transpose x to [in_f, tokens]; expand INVERTED sign bits to
# uint16 masks {0, 0x8000} laid out [i, (b, o)] so the XOR is one linear op.

import sys

import numpy as np

for p in ("/opt/trn_rl_repo", "/opt/pypackages"):
    if p not in sys.path:
        sys.path.insert(0, p)

import concourse.bacc as bacc
import concourse.mybir as mybir
import concourse.tile as tile
from concourse.bass_utils import run_bass_kernel_spmd

W_BIT, OUT_F, IN_F, K = 4, 4096, 4096, 16
B, S = 4, 2048
T = B * S                      # 8192 tokens
NCORES = 8
OS = OUT_F // NCORES           # 512 out features per core
N_ITILES = IN_F // 128         # 32

# token groups: (start_token, n_ttiles). group 0 runs under formation with 4
# psum banks (the other 4 hold the in-flight low-rank psums); the rest use 8.
GROUPS = [(0, 4)] + [(512 + 1024 * g, 8) for g in range(7)] + [(7680, 4)]

FP16 = mybir.dt.float16
F32 = mybir.dt.float32
U16 = mybir.dt.uint16
Alu = mybir.AluOpType

_cached = {}


def build_nc():
    nc = bacc.Bacc("TRN2", target_bir_lowering=False, debug=False,
                   num_devices=NCORES)
    xt_p = nc.dram_tensor("xt", [IN_F, T], FP16, kind="ExternalInput").ap()
    qp_p = nc.dram_tensor("qm", [IN_F, W_BIT * OS], U16,
                          kind="ExternalInput").ap()
    ut_p = nc.dram_tensor("ut", [W_BIT, K, OS], FP16, kind="ExternalInput").ap()
    vt_p = nc.dram_tensor("vt4", [W_BIT, K, IN_F], FP16, kind="ExternalInput").ap()
    out_p = nc.dram_tensor("out", [T, OS], FP16, kind="ExternalOutput").ap()

    with tile.TileContext(nc) as tc:
        with (
            tc.tile_pool(name="const", bufs=1) as cpool,
            tc.tile_pool(name="wt", bufs=1) as wtpool,
            tc.tile_pool(name="fls", bufs=2) as fls,
            tc.tile_pool(name="fmk", bufs=3) as fmk,
            tc.tile_pool(name="fpr", bufs=2) as fpr,
            tc.tile_pool(name="fp01", bufs=2) as fp01,
            tc.tile_pool(name="mx", bufs=8) as mx,
            tc.tile_pool(name="mo", bufs=8) as mo,
        ):
            # resident operands
            vt_b = []
            ut_b = []
            for b in range(W_BIT):
                v = cpool.tile([K, IN_F], FP16, tag=f"vt{b}")
                nc.sync.dma_start(v[:], vt_p[b, :, :])
                vt_b.append(v)
                uu = cpool.tile([K, OS], FP16, tag=f"ut{b}")
                nc.sync.dma_start(uu[:], ut_p[b, :, :])
                ut_b.append(uu)

            # w.T tiles, one per i-tile
            wts = [
                wtpool.tile([128, OS], FP16, tag=f"wt{it}", name=f"wt_{it}")
                for it in range(N_ITILES)
            ]

            def mm_group(gi, it):
                t0, ntt = GROUPS[gi]
                xs = mx.tile([128, ntt * 128], FP16, tag="x")
                nc.sync.dma_start(
                    xs[:], xt_p[it * 128:(it + 1) * 128, t0:t0 + ntt * 128]
                )
                for tt in range(ntt):
                    nc.tensor.matmul(
                        acc_tiles[tt][:],
                        xs[:, tt * 128:(tt + 1) * 128],
                        wts[it][:],
                        start=(it == 0),
                        stop=(it == N_ITILES - 1),
                    )

            def flush_group(gi):
                t0, ntt = GROUPS[gi]
                for tt in range(ntt):
                    ot = mo.tile([128, OS], FP16, tag="o")
                    if tt % 2 == 0:
                        nc.scalar.copy(ot[:], acc_tiles[tt][:])
                    else:
                        nc.vector.tensor_copy(ot[:], acc_tiles[tt][:])
                    r0 = t0 + tt * 128
                    nc.sync.dma_start(out_p[r0:r0 + 128, :], ot[:])

            # ---- formation (per i-tile) pipelined with token group 0 ----
            with (
                tc.tile_pool(name="mps0", bufs=4, space="PSUM") as mps0,
                tc.tile_pool(name="psL", bufs=2, space="PSUM") as psL,
            ):
                acc_tiles = [
                    mps0.tile([128, OS], F32, tag="ps", name=f"acc_0_{tt}")
                    for tt in range(GROUPS[0][1])
                ]
                for it in range(N_ITILES):
                    isl = slice(it * 128, it * 128 + 128)

                    # host-precomputed sign masks {0, 0x8000} for this i-tile
                    mk = fmk.tile([128, W_BIT * OS], U16, tag="mk",
                                  name=f"mk_{it}")
                    nc.sync.dma_start(mk[:], qp_p[isl, :])

                    # low-rank psums, planes 01 -> plA, planes 23 -> plB
                    plA = psL.tile([128, 2 * OS], F32, tag="pl",
                                   name=f"plA_{it}")
                    nc.tensor.matmul(plA[:, 0:OS], vt_b[0][:, isl], ut_b[0][:],
                                     start=True, stop=True)
                    nc.tensor.matmul(plA[:, OS:2 * OS], vt_b[1][:, isl],
                                     ut_b[1][:], start=True, stop=True)
                    plB = psL.tile([128, 2 * OS], F32, tag="pl",
                                   name=f"plB_{it}")
                    nc.tensor.matmul(plB[:, 0:OS], vt_b[2][:, isl], ut_b[2][:],
                                     start=True, stop=True)
                    nc.tensor.matmul(plB[:, OS:2 * OS], vt_b[3][:, isl],
                                     ut_b[3][:], start=True, stop=True)

                    # evacuate to fp16 (Scalar)
                    ls = fls.tile([128, W_BIT * OS], FP16, tag="ls")
                    nc.scalar.copy(ls[:, 0:2 * OS], plA[:])
                    nc.scalar.copy(ls[:, 2 * OS:4 * OS], plB[:])

                    # prods = ls ^ masks (flips fp16 sign bit -> exact +-L)
                    pr = fpr.tile([128, W_BIT * OS], FP16, tag="pr")
                    nc.vector.tensor_tensor(
                        pr[:].bitcast(U16), ls[:].bitcast(U16), mk[:],
                        op=Alu.bitwise_xor,
                    )

                    # wT = (p0+p2) + (p1+p3), both adds on DVE
                    p01 = fp01.tile([128, 2 * OS], FP16, tag="p01")
                    nc.vector.tensor_add(
                        p01[:], pr[:, 0:2 * OS], pr[:, 2 * OS:4 * OS]
                    )
                    nc.vector.tensor_add(
                        wts[it][:], p01[:, 0:OS], p01[:, OS:2 * OS]
                    )
                    mm_group(0, it)
                flush_group(0)

            # ---- remaining token groups (full 8 psum banks) ----
            with tc.tile_pool(name="mps", bufs=8, space="PSUM") as mps:
                for gi in range(1, len(GROUPS)):
                    acc_tiles = [
                        mps.tile([128, OS], F32, tag="ps", name=f"acc_{gi}_{tt}")
                        for tt in range(GROUPS[gi][1])
                    ]
                    for it in range(N_ITILES):
                        mm_group(gi, it)
                    flush_group(gi)
    nc.compile()
    return nc


def prep_inputs(x, qweight, u, vt):
    """Host-side shard prep. Returns per-core input maps."""
    x = np.asarray(x, dtype=np.float16)
    qweight = np.asarray(qweight)
    u = np.asarray(u, dtype=np.float16)
    vt = np.ascontiguousarray(np.asarray(vt, dtype=np.float16))

    xt = np.ascontiguousarray(x.reshape(T, IN_F).T)  # [IN_F, T]

    # unpack bits: (b, o, i); INVERT so mask=0x8000 <=> sign -1 (bit 0)
    bytes_ = qweight.astype(np.uint8)
    bits = np.unpackbits(bytes_.reshape(W_BIT, -1, 1), axis=2, bitorder="little")
    bits = bits.reshape(W_BIT, OUT_F, IN_F)
    # per core c: mask[i, b*OS + o] = inv(b, o_global=c*OS+o, i) << 15
    inv = (1 - bits.astype(np.uint16)) << np.uint16(15)  # [b, o, i]
    iv = inv.reshape(W_BIT, NCORES, OS, IN_F)       # [b, c, o, i]
    qm_all = iv.transpose(1, 3, 0, 2)               # [c, i, b, o]
    qm_all = np.ascontiguousarray(qm_all).reshape(NCORES, IN_F, W_BIT * OS)

    in_maps = []
    for c in range(NCORES):
        uc = u[:, c * OS:(c + 1) * OS, :]                 # [4, 512, 16]
        ut = np.ascontiguousarray(uc.transpose(0, 2, 1))  # [4, 16, 512]
        in_maps.append({"xt": xt, "qm": qm_all[c], "ut": ut, "vt4": vt})
    return in_maps


def kernel(x, qweight, u, vt, _trace=False):
    if "nc" not in _cached:
        _cached["nc"] = build_nc()
    nc = _cached["nc"]
    in_maps = prep_inputs(x, qweight, u, vt)
    res = run_bass_kernel_spmd(nc, in_maps, list(range(NCORES)), trace=_trace)
    _cached["last_result"] = res
    out = np.concatenate([res.results[c]["out"] for c in range(NCORES)], axis=1)
    return out.reshape(B, S, OUT_F).astype(np.float16)
